# revision 25
# baseline (speedup 1.0000x reference)
"""GQA causal-attention prefill kernel for Trainium2 (8 NeuronCores), v4.

Sharding: head-parallel - core c gets query heads 4c..4c+3 and kv head c.

Per-core device algorithm (matmuls bf16, scores transposed [key, q]):
  S^T[key, q] = kT_blk.T @ qT_blk                  (PE, exact causal widths)
  P^T = exp(SCALE * S^T) split across engines:
    - non-diag groups: ACT real exp, or Pool Schraudolph bit-trick
    - diag groups: DVE/Pool scalar_tensor_tensor fused Schraudolph+mask:
        i16 = rint(S*A + Bmask),  Bmask = B_S (live) / B_S-4000 (masked)
      masked entries decode to ~2^-25 in bf16 -> effectively zero, so no
      separate mask multiplies are needed anywhere.
  outT[d, q] += V_blk.T @ P^T_blk                  (PE, PSUM accumulate)
  pair[key, q] = P^T_blk0 + P^T_blk1 ship slots    (DVE adds/copies)
  out-copy PSUM->SBUF bf16                         (ACT)
  DMA out: unnormalized outT (bf16) + per-(h,M) ship tile of pair sums

(h, M) processed M-descending so the kernel ends on the small M=0 chunk
(short tail).  Host: l[q] = sum over shipped pair rows; out = outT / l.
"""

import numpy as np
import ml_dtypes

BF16 = ml_dtypes.bfloat16

SEQ = 2048
NUM_HEADS = 32
NUM_KV_HEADS = 8
D = 128
NCORES = 8
HPC = NUM_HEADS // NCORES
SCALE = float(1.0 / np.sqrt(D))

P = 128
QB = 512
NQB = SEQ // QB
NKB = SEQ // P
NSLOT = 2 * NQB  # max pair slots per (h, M): 2M+2 <= 8

A_S = float(SCALE * 128.0 / np.log(2.0))
B_S = 16255.0
B_MASKED = B_S - 4000.0  # decodes to ~2^-25: effectively zero post-bitcast

_COMPILED = {}


def _groups():
    """Global group order: per head M descending (3,2,1,0), but the small
    M=1 / M=0 chunks are woven into the NEXT head's M=3 stream so PE always
    has independent QK work while their exps drain.  At most 2 chunks are
    ever open simultaneously (accpool bufs=2)."""
    def chunk(h, M):
        npairs = 2 * (M + 1)
        return [(h, M, gp, gp == 0, gp == npairs - 1) for gp in range(npairs)]

    def weave(a, b):
        # a: earlier chunk's remaining groups, b: next chunk's groups.
        # alternate 1:1 starting with a, then append the rest.
        out = []
        ia = ib = 0
        while ia < len(a) or ib < len(b):
            if ia < len(a):
                out.append(a[ia]); ia += 1
            if ib < len(b):
                out.append(b[ib]); ib += 1
        return out

    gs = []
    carry_prev = []
    for h in range(HPC):
        gs += weave(carry_prev, chunk(h, 3))
        if h == HPC - 1:
            # last head: plain order — chunks close spread out, so their
            # ship/outT DMAs don't pile up on HWDGE at the very end.  The
            # 6-deep st pool keeps PE fed without weaving here.
            gs += chunk(h, 2) + chunk(h, 1) + chunk(h, 0)
            carry_prev = []
        else:
            gs += chunk(h, 2)
            carry_prev = chunk(h, 1) + chunk(h, 0)
    return gs


def _ne_engine(M, gp, t):
    # non-diagonal exp engine per half-group (2*gp+t): ACT ~60 / Pool ~36
    pat = {3: "APAAPAAPAPAP", 2: "APAAPAAP", 1: "APAA"}[M]
    return pat[2 * gp + t]


def _build(num_devices=NCORES, reps=1):
    import concourse.mybir as mybir
    import concourse.tile as tile
    from concourse import bacc

    f32 = mybir.dt.float32
    bf16 = mybir.dt.bfloat16
    i16 = mybir.dt.int16
    Exp = mybir.ActivationFunctionType.Exp
    Copy = mybir.ActivationFunctionType.Copy
    MULT = mybir.AluOpType.mult
    ADD = mybir.AluOpType.add

    nc = bacc.Bacc(
        "TRN2", target_bir_lowering=False, debug=False, num_devices=num_devices
    )

    qT_d = nc.dram_tensor("qT", [HPC, P, SEQ], bf16, kind="ExternalInput")
    kT_d = nc.dram_tensor("kT", [P, SEQ], bf16, kind="ExternalInput")
    v_d = nc.dram_tensor("v", [P, SEQ], bf16, kind="ExternalInput")
    bm_d = nc.dram_tensor("bmask", [P, 2 * QB], i16, kind="ExternalInput")
    outT_d = nc.dram_tensor("outT", [HPC, NQB, P, QB], bf16, kind="ExternalOutput")
    # ship layout: [h, M, partition(key), slot, q] so SBUF [p, slot, q] maps 1:1
    acc_d = nc.dram_tensor("acc", [HPC, NQB, P, NSLOT, QB], bf16,
                           kind="ExternalOutput")

    with tile.TileContext(nc) as tc:
        with (
            tc.tile_pool(name="const", bufs=1) as cpool,
            tc.tile_pool(name="pt", bufs=10, space="SBUF") as ptpool,
            tc.tile_pool(name="ship", bufs=2) as shpool,
            tc.tile_pool(name="ob", bufs=3) as obpool,
            tc.tile_pool(name="st", bufs=6, space="PSUM") as stpool,
            tc.tile_pool(name="acc", bufs=2, space="PSUM") as accpool,
        ):
            # kT / v split in three chunks so early blocks land fast
            kT_sb = [
                cpool.tile([P, QB], bf16, tag="kT0", name="kT_sb0"),
                cpool.tile([P, QB], bf16, tag="kT1", name="kT_sb1"),
                cpool.tile([P, 2 * QB], bf16, tag="kT2", name="kT_sb2"),
            ]
            v_sb = [
                cpool.tile([P, QB], bf16, tag="v0", name="v_sb0"),
                cpool.tile([P, QB], bf16, tag="v1", name="v_sb1"),
                cpool.tile([P, 2 * QB], bf16, tag="v2", name="v_sb2"),
            ]
            # q: head 0 split (M=3 chunk first since M descends), rest whole
            qA0 = cpool.tile([P, QB], bf16, tag="qA0")
            qB0 = cpool.tile([P, 3 * QB], bf16, tag="qB0")
            q_sb = [
                cpool.tile([P, SEQ], bf16, tag=f"q{h}", name=f"q_sb{h}")
                for h in range(1, HPC)
            ]
            bm_sb = cpool.tile([P, 2 * QB], i16, tag="bm")
            warm_sb = cpool.tile([P, 1], f32, tag="warm")

            # input DMAs: critical-first.  q loads for head 0 go on the ACT
            # queue (idle at t=0); everything else on the SP queue.
            nc.sync.dma_start(kT_sb[0][:], kT_d.ap()[:, 0:QB])
            nc.scalar.dma_start(qA0[:], qT_d.ap()[0][:, 3 * QB : 4 * QB])
            nc.vector.memset(warm_sb[:], 0.0)
            nc.scalar.activation(warm_sb[:], warm_sb[:], Exp, scale=SCALE)
            nc.sync.dma_start(kT_sb[1][:], kT_d.ap()[:, QB : 2 * QB])
            nc.sync.dma_start(bm_sb[:], bm_d.ap())
            nc.sync.dma_start(v_sb[0][:], v_d.ap()[:, 0:QB])
            nc.sync.dma_start(kT_sb[2][:], kT_d.ap()[:, 2 * QB : 4 * QB])
            nc.sync.dma_start(v_sb[1][:], v_d.ap()[:, QB : 2 * QB])
            nc.sync.dma_start(v_sb[2][:], v_d.ap()[:, 2 * QB : 4 * QB])
            nc.sync.dma_start(qB0[:], qT_d.ap()[0][:, 0 : 3 * QB])
            for h in range(1, HPC):
                nc.sync.dma_start(q_sb[h - 1][:], qT_d.ap()[h])

            def kT_blk(j):
                if j < 4:
                    return kT_sb[0][:, (j % 4) * P : (j % 4 + 1) * P]
                if j < 8:
                    return kT_sb[1][:, (j % 4) * P : (j % 4 + 1) * P]
                return kT_sb[2][:, (j - 8) * P : (j - 7) * P]

            def v_blk(j):
                if j < 4:
                    return v_sb[0][:, (j % 4) * P : (j % 4 + 1) * P]
                if j < 8:
                    return v_sb[1][:, (j % 4) * P : (j % 4 + 1) * P]
                return v_sb[2][:, (j - 8) * P : (j - 7) * P]

            def q_tile(h, M):
                if h == 0:
                    return qA0[:] if M == 3 else qB0[:, M * QB : (M + 1) * QB]
                return q_sb[h - 1][:, M * QB : (M + 1) * QB]

            groups = _groups()
            last_idx = len(groups) - 1

            state = {}

            def produce(idx):
                h, M, gp, _, _ = groups[idx]
                rep = state.get("rep", 0)
                # one single-bank PSUM tile per key block (half-group) so PE
                # can run 6 blocks ahead of exp completion
                pt = ptpool.tile([P, 2 * QB], bf16, tag="pt", name=f"pt{rep}_{idx}")
                for t in range(2):
                    j = 2 * gp + t
                    u = j - 4 * M
                    lo = u * P if u > 0 else 0
                    st = stpool.tile(
                        [P, QB], f32, tag="st", name=f"st{rep}_{idx}_{t}"
                    )
                    nc.tensor.matmul(
                        st[:, lo:QB],
                        lhsT=kT_blk(j),
                        rhs=q_tile(h, M)[:, lo:QB],
                        start=True,
                        stop=True,
                    )
                    po = t * QB  # pt column offset for this half
                    if gp == 2 * M:
                        # diag: fused Schraudolph exp + causal mask on DVE
                        nc.vector.scalar_tensor_tensor(
                            pt[:, po : po + QB].bitcast(i16), st[:], A_S,
                            bm_sb[:, t * QB : (t + 1) * QB],
                            op0=MULT, op1=ADD,
                        )
                    elif gp == 2 * M + 1:
                        # diag B: live cols [2P:QB); same bmask pattern shifted
                        eng = nc.vector if idx == last_idx else nc.gpsimd
                        eng.scalar_tensor_tensor(
                            pt[:, po + 2 * P : po + QB].bitcast(i16),
                            st[:, 2 * P :],
                            A_S,
                            bm_sb[:, t * QB : t * QB + 2 * P],
                            op0=MULT, op1=ADD,
                        )
                    else:
                        if _ne_engine(M, gp, t) == "A":
                            nc.scalar.activation(
                                pt[:, po : po + QB], st[:], Exp, scale=SCALE
                            )
                        else:
                            nc.gpsimd.tensor_scalar(
                                pt[:, po : po + QB].bitcast(i16), st[:],
                                A_S, B_S, op0=MULT, op1=ADD,
                            )
                state[idx] = pt

            def consume(idx):
                h, M, gp, first, last = groups[idx]
                rep = state.get("rep", 0)
                pt = state.pop(idx)
                if first:
                    state["out_ps", h, M] = accpool.tile(
                        [P, QB], f32, tag="out", name=f"out{rep}_{h}_{M}"
                    )
                    state["ship", h, M] = shpool.tile(
                        [P, 2 * M + 2, QB], bf16, tag=f"ship{M}", name=f"sh{rep}_{h}_{M}"
                    )
                out_ps = state["out_ps", h, M]
                ship = state["ship", h, M]
                for t in range(2):
                    j = 2 * gp + t
                    u = j - 4 * M
                    lo = u * P if u > 0 else 0
                    nc.tensor.matmul(
                        out_ps[:, lo:QB],
                        lhsT=v_blk(j),
                        rhs=pt[:, t * QB + lo : (t + 1) * QB],
                        start=(first and t == 0),
                        stop=(last and t == 1),
                    )
                tail_split = False
                copy_split = False
                if gp == 2 * M:
                    # pair A -> slot 2M: [0,P) only block0; add the rest
                    nc.vector.tensor_copy(ship[:, gp, 0:P], pt[:, 0:P])
                    nc.vector.tensor_add(
                        ship[:, gp, P:QB], pt[:, P:QB], pt[:, QB + P : 2 * QB]
                    )
                    if tail_split:
                        nc.sync.dma_start(
                            acc_d.ap()[h][M][:, 2 * M : 2 * M + 1, :],
                            ship[:, 2 * M : 2 * M + 1, :],
                        )
                    if copy_split:
                        # cols [0, 2P) of out_ps are final after diag-A PVs:
                        # copy+ship them while diag-B still accumulates hi cols
                        osb = obpool.tile(
                            [P, QB], bf16, tag="ob", name=f"ob{rep}_{h}{M}"
                        )
                        state["osb", h, M] = osb
                        nc.scalar.activation(
                            osb[:, 0 : 2 * P], out_ps[:, 0 : 2 * P], Copy
                        )
                        nc.scalar.dma_start(
                            outT_d.ap()[h][M][:, 0 : 2 * P], osb[:, 0 : 2 * P]
                        )
                elif gp == 2 * M + 1:
                    # pair B -> slot 2M+1, live [2P, QB); host ignores [0, 2P)
                    nc.vector.tensor_copy(
                        ship[:, gp, 2 * P : 3 * P], pt[:, 2 * P : 3 * P]
                    )
                    nc.vector.tensor_add(
                        ship[:, gp, 3 * P : QB],
                        pt[:, 3 * P : QB],
                        pt[:, QB + 3 * P : 2 * QB],
                    )
                    lo_slot = 2 * M + 1 if tail_split else 2 * M
                    nc.sync.dma_start(
                        acc_d.ap()[h][M][:, lo_slot : 2 * M + 2, :],
                        ship[:, lo_slot : 2 * M + 2, :],
                    )
                else:
                    nc.vector.tensor_add(
                        ship[:, gp], pt[:, 0:QB], pt[:, QB : 2 * QB]
                    )
                    if gp == 2 * M - 1:
                        # bulk ship: all non-diag slots [0, 2M) complete
                        nc.sync.dma_start(
                            acc_d.ap()[h][M][:, 0 : 2 * M, :],
                            ship[:, 0 : 2 * M, :],
                        )

                if last:
                    if copy_split:
                        osb = state.pop(("osb", h, M))
                        nc.scalar.activation(
                            osb[:, 2 * P :], out_ps[:, 2 * P :], Copy
                        )
                        nc.scalar.dma_start(
                            outT_d.ap()[h][M][:, 2 * P :], osb[:, 2 * P :]
                        )
                    else:
                        osb = obpool.tile(
                            [P, QB], bf16, tag="ob", name=f"ob{rep}_{h}{M}"
                        )
                        nc.scalar.activation(osb[:], out_ps[:], Copy)
                        nc.scalar.dma_start(outT_d.ap()[h][M], osb[:])
                    del state["out_ps", h, M]
                    del state["ship", h, M]

            LOOKAHEAD = 8
            for rep in range(reps):
                state["rep"] = rep
                for i in range(min(LOOKAHEAD, len(groups))):
                    produce(i)
                for i in range(len(groups)):
                    if i + LOOKAHEAD < len(groups):
                        produce(i + LOOKAHEAD)
                    consume(i)

    nc.compile()
    return nc


def _host_bmask():
    p = np.arange(P)[:, None, None]
    t = np.arange(2)[None, :, None]
    c = np.arange(QB)[None, None, :]
    bm = np.where(c >= 128 * t + p, B_S, B_MASKED).astype(np.int16)
    return bm.reshape(P, 2 * QB)


def _pack_inputs(q, k, v):
    """Per-core input dict list (shared by kernel() and test harnesses)."""
    bm = _host_bmask()
    in_maps = []
    for c in range(NCORES):
        qT_c = np.ascontiguousarray(
            q[:, HPC * c : HPC * (c + 1), :].transpose(1, 2, 0)
        ).astype(BF16)
        kT_c = np.ascontiguousarray(k[:, c, :].T).astype(BF16)
        v_c = np.ascontiguousarray(
            v[:, c, :].reshape(NKB, P, D).transpose(1, 0, 2).reshape(P, SEQ)
        ).astype(BF16)
        in_maps.append({"qT": qT_c, "kT": kT_c, "v": v_c, "bmask": bm})
    return in_maps


def kernel(q, k, v, k_cache=None, v_cache=None, slot_mapping=None, **_):
    from concourse.bass_utils import run_bass_kernel_spmd

    if "nc" not in _COMPILED:
        _COMPILED["nc"] = _build()
    nc = _COMPILED["nc"]

    q = np.asarray(q, dtype=np.float32)
    k = np.asarray(k, dtype=np.float32)
    v = np.asarray(v, dtype=np.float32)

    in_maps = _pack_inputs(q, k, v)
    res = run_bass_kernel_spmd(nc, in_maps, list(range(NCORES)))

    out = np.empty((SEQ, NUM_HEADS, D), np.float32)
    for c in range(NCORES):
        oT = res.results[c]["outT"].astype(np.float32)   # [HPC, NQB, d, q]
        ac = res.results[c]["acc"]                        # [HPC, NQB, p, slot, q]
        for h in range(HPC):
            for M in range(NQB):
                a = ac[h, M].astype(np.float32)  # [128, NSLOT, 512]
                l = a[:, 0 : 2 * M + 1, :].sum(axis=(0, 1))  # pairs + pair A
                l[2 * P :] += a[:, 2 * M + 1, 2 * P :].sum(axis=0)  # pair B
                out[M * QB : (M + 1) * QB, HPC * c + h, :] = (oT[h, M] / l).T
    return out


# revision 26
# speedup vs baseline: 1.0404x; 1.0404x over previous
"""GQA causal-attention prefill kernel for Trainium2 (8 NeuronCores), v4.

Sharding: head-parallel - core c gets query heads 4c..4c+3 and kv head c.

Per-core device algorithm (matmuls bf16, scores transposed [key, q]):
  S^T[key, q] = kT_blk.T @ qT_blk                  (PE, exact causal widths)
  P^T = exp(SCALE * S^T) split across engines:
    - non-diag groups: ACT real exp, or Pool Schraudolph bit-trick
    - diag groups: DVE/Pool scalar_tensor_tensor fused Schraudolph+mask:
        i16 = rint(S*A + Bmask),  Bmask = B_S (live) / B_S-4000 (masked)
      masked entries decode to ~2^-25 in bf16 -> effectively zero, so no
      separate mask multiplies are needed anywhere.
  outT[d, q] += V_blk.T @ P^T_blk                  (PE, PSUM accumulate)
  pair[key, q] = P^T_blk0 + P^T_blk1 ship slots    (DVE adds/copies)
  out-copy PSUM->SBUF bf16                         (ACT)
  DMA out: unnormalized outT (bf16) + per-(h,M) ship tile of pair sums

(h, M) processed M-descending so the kernel ends on the small M=0 chunk
(short tail).  Host: l[q] = sum over shipped pair rows; out = outT / l.
"""

import numpy as np
import ml_dtypes

BF16 = ml_dtypes.bfloat16

SEQ = 2048
NUM_HEADS = 32
NUM_KV_HEADS = 8
D = 128
NCORES = 8
HPC = NUM_HEADS // NCORES
SCALE = float(1.0 / np.sqrt(D))

P = 128
QB = 512
NQB = SEQ // QB
NKB = SEQ // P
NSLOT = 2 * NQB  # max pair slots per (h, M): 2M+2 <= 8

A_S = float(SCALE * 128.0 / np.log(2.0))
B_S = 16255.0
B_MASKED = B_S - 4000.0  # decodes to ~2^-25: effectively zero post-bitcast

_COMPILED = {}


def _groups():
    """Global group order: per head M descending (3,2,1,0), but the small
    M=1 / M=0 chunks are woven into the NEXT head's M=3 stream so PE always
    has independent QK work while their exps drain.  At most 2 chunks are
    ever open simultaneously (accpool bufs=2)."""
    def chunk(h, M):
        npairs = 2 * (M + 1)
        return [(h, M, gp, gp == 0, gp == npairs - 1) for gp in range(npairs)]

    def weave(a, b):
        # a: earlier chunk's remaining groups, b: next chunk's groups.
        # alternate 1:1 starting with a, then append the rest.
        out = []
        ia = ib = 0
        while ia < len(a) or ib < len(b):
            if ia < len(a):
                out.append(a[ia]); ia += 1
            if ib < len(b):
                out.append(b[ib]); ib += 1
        return out

    gs = []
    carry_prev = []
    for h in range(HPC):
        gs += weave(carry_prev, chunk(h, 3))
        if h == HPC - 1:
            # last head: plain order — chunks close spread out, so their
            # ship/outT DMAs don't pile up on HWDGE at the very end.  The
            # 6-deep st pool keeps PE fed without weaving here.
            gs += chunk(h, 2) + chunk(h, 1) + chunk(h, 0)
            carry_prev = []
        else:
            gs += chunk(h, 2)
            carry_prev = chunk(h, 1) + chunk(h, 0)
    return gs


def _ne_engine(M, gp, t):
    # non-diagonal exp engine per half-group (2*gp+t): ACT ~60 / Pool ~36
    pat = {3: "APAAPAAPAPAP", 2: "APAAPAAP", 1: "APAA"}[M]
    return pat[2 * gp + t]


def _build(num_devices=NCORES, reps=1):
    import concourse.mybir as mybir
    import concourse.tile as tile
    from concourse import bacc

    f32 = mybir.dt.float32
    bf16 = mybir.dt.bfloat16
    i16 = mybir.dt.int16
    Exp = mybir.ActivationFunctionType.Exp
    Copy = mybir.ActivationFunctionType.Copy
    MULT = mybir.AluOpType.mult
    ADD = mybir.AluOpType.add

    nc = bacc.Bacc(
        "TRN2", target_bir_lowering=False, debug=False, num_devices=num_devices
    )

    qT_d = nc.dram_tensor("qT", [HPC, P, SEQ], bf16, kind="ExternalInput")
    kT_d = nc.dram_tensor("kT", [P, SEQ], bf16, kind="ExternalInput")
    v_d = nc.dram_tensor("v", [P, SEQ], bf16, kind="ExternalInput")
    bm_d = nc.dram_tensor("bmask", [P, 2 * QB], i16, kind="ExternalInput")
    outT_d = nc.dram_tensor("outT", [HPC, NQB, P, QB], bf16, kind="ExternalOutput")
    # ship layout: [h, M, partition(key), slot, q] so SBUF [p, slot, q] maps 1:1
    acc_d = nc.dram_tensor("acc", [HPC, NQB, P, NSLOT, QB], bf16,
                           kind="ExternalOutput")

    with tile.TileContext(nc) as tc:
        with (
            tc.tile_pool(name="const", bufs=1) as cpool,
            tc.tile_pool(name="pt", bufs=10, space="SBUF") as ptpool,
            tc.tile_pool(name="ship", bufs=2) as shpool,
            tc.tile_pool(name="ob", bufs=3) as obpool,
            tc.tile_pool(name="st", bufs=6, space="PSUM") as stpool,
            tc.tile_pool(name="acc", bufs=2, space="PSUM") as accpool,
        ):
            # kT / v split in three chunks so early blocks land fast
            kT_sb = [
                cpool.tile([P, QB], bf16, tag="kT0", name="kT_sb0"),
                cpool.tile([P, QB], bf16, tag="kT1", name="kT_sb1"),
                cpool.tile([P, 2 * QB], bf16, tag="kT2", name="kT_sb2"),
            ]
            v_sb = [
                cpool.tile([P, QB], bf16, tag="v0", name="v_sb0"),
                cpool.tile([P, QB], bf16, tag="v1", name="v_sb1"),
                cpool.tile([P, 2 * QB], bf16, tag="v2", name="v_sb2"),
            ]
            # q: head 0 split (M=3 chunk first since M descends), rest whole
            qA0 = cpool.tile([P, QB], bf16, tag="qA0")
            qB0 = cpool.tile([P, 3 * QB], bf16, tag="qB0")
            q_sb = [
                cpool.tile([P, SEQ], bf16, tag=f"q{h}", name=f"q_sb{h}")
                for h in range(1, HPC)
            ]
            bm_sb = cpool.tile([P, 2 * QB], i16, tag="bm")
            warm_sb = cpool.tile([P, 1], f32, tag="warm")

            # input DMAs: critical-first.  q loads for head 0 go on the ACT
            # queue (idle at t=0); everything else on the SP queue.
            nc.sync.dma_start(kT_sb[0][:], kT_d.ap()[:, 0:QB])
            nc.scalar.dma_start(qA0[:], qT_d.ap()[0][:, 3 * QB : 4 * QB])
            nc.vector.memset(warm_sb[:], 0.0)
            nc.scalar.activation(warm_sb[:], warm_sb[:], Exp, scale=SCALE)
            nc.sync.dma_start(kT_sb[1][:], kT_d.ap()[:, QB : 2 * QB])
            nc.sync.dma_start(bm_sb[:], bm_d.ap())
            nc.sync.dma_start(v_sb[0][:], v_d.ap()[:, 0:QB])
            nc.sync.dma_start(kT_sb[2][:], kT_d.ap()[:, 2 * QB : 4 * QB])
            nc.sync.dma_start(v_sb[1][:], v_d.ap()[:, QB : 2 * QB])
            nc.sync.dma_start(v_sb[2][:], v_d.ap()[:, 2 * QB : 4 * QB])
            nc.sync.dma_start(qB0[:], qT_d.ap()[0][:, 0 : 3 * QB])
            for h in range(1, HPC):
                nc.sync.dma_start(q_sb[h - 1][:], qT_d.ap()[h])

            def kT_blk(j):
                if j < 4:
                    return kT_sb[0][:, (j % 4) * P : (j % 4 + 1) * P]
                if j < 8:
                    return kT_sb[1][:, (j % 4) * P : (j % 4 + 1) * P]
                return kT_sb[2][:, (j - 8) * P : (j - 7) * P]

            def v_blk(j):
                if j < 4:
                    return v_sb[0][:, (j % 4) * P : (j % 4 + 1) * P]
                if j < 8:
                    return v_sb[1][:, (j % 4) * P : (j % 4 + 1) * P]
                return v_sb[2][:, (j - 8) * P : (j - 7) * P]

            def q_tile(h, M):
                if h == 0:
                    return qA0[:] if M == 3 else qB0[:, M * QB : (M + 1) * QB]
                return q_sb[h - 1][:, M * QB : (M + 1) * QB]

            groups = _groups()
            last_idx = len(groups) - 1

            state = {}

            def produce(idx):
                h, M, gp, _, _ = groups[idx]
                rep = state.get("rep", 0)
                # one single-bank PSUM tile per key block (half-group) so PE
                # can run 6 blocks ahead of exp completion
                pt = ptpool.tile([P, 2 * QB], bf16, tag="pt", name=f"pt{rep}_{idx}")
                for t in range(2):
                    j = 2 * gp + t
                    u = j - 4 * M
                    lo = u * P if u > 0 else 0
                    st = stpool.tile(
                        [P, QB], f32, tag="st", name=f"st{rep}_{idx}_{t}"
                    )
                    nc.tensor.matmul(
                        st[:, lo:QB],
                        lhsT=kT_blk(j),
                        rhs=q_tile(h, M)[:, lo:QB],
                        start=True,
                        stop=True,
                    )
                    po = t * QB  # pt column offset for this half
                    if gp == 2 * M:
                        # diag: fused Schraudolph exp + causal mask on DVE
                        nc.vector.scalar_tensor_tensor(
                            pt[:, po : po + QB].bitcast(i16), st[:], A_S,
                            bm_sb[:, t * QB : (t + 1) * QB],
                            op0=MULT, op1=ADD,
                        )
                    elif gp == 2 * M + 1:
                        # diag B: live cols [2P:QB); same bmask pattern shifted
                        eng = nc.vector if idx == last_idx else nc.gpsimd
                        eng.scalar_tensor_tensor(
                            pt[:, po + 2 * P : po + QB].bitcast(i16),
                            st[:, 2 * P :],
                            A_S,
                            bm_sb[:, t * QB : t * QB + 2 * P],
                            op0=MULT, op1=ADD,
                        )
                    else:
                        if _ne_engine(M, gp, t) == "A":
                            nc.scalar.activation(
                                pt[:, po : po + QB], st[:], Exp, scale=SCALE
                            )
                        else:
                            nc.gpsimd.tensor_scalar(
                                pt[:, po : po + QB].bitcast(i16), st[:],
                                A_S, B_S, op0=MULT, op1=ADD,
                            )
                state[idx] = pt

            def consume(idx):
                h, M, gp, first, last = groups[idx]
                rep = state.get("rep", 0)
                pt = state.pop(idx)
                if first:
                    state["out_ps", h, M] = accpool.tile(
                        [P, QB], f32, tag="out", name=f"out{rep}_{h}_{M}"
                    )
                    state["ship", h, M] = shpool.tile(
                        [P, 2 * M + 2, QB], bf16, tag=f"ship{M}", name=f"sh{rep}_{h}_{M}"
                    )
                out_ps = state["out_ps", h, M]
                ship = state["ship", h, M]
                for t in range(2):
                    j = 2 * gp + t
                    u = j - 4 * M
                    lo = u * P if u > 0 else 0
                    nc.tensor.matmul(
                        out_ps[:, lo:QB],
                        lhsT=v_blk(j),
                        rhs=pt[:, t * QB + lo : (t + 1) * QB],
                        start=(first and t == 0),
                        stop=(last and t == 1),
                    )
                tail_split = False
                copy_split = False
                if gp == 2 * M:
                    # pair A -> slot 2M: [0,P) only block0; add the rest
                    nc.vector.tensor_copy(ship[:, gp, 0:P], pt[:, 0:P])
                    nc.vector.tensor_add(
                        ship[:, gp, P:QB], pt[:, P:QB], pt[:, QB + P : 2 * QB]
                    )
                    if tail_split:
                        nc.sync.dma_start(
                            acc_d.ap()[h][M][:, 2 * M : 2 * M + 1, :],
                            ship[:, 2 * M : 2 * M + 1, :],
                        )
                    if copy_split:
                        # cols [0, 2P) of out_ps are final after diag-A PVs:
                        # copy+ship them while diag-B still accumulates hi cols
                        osb = obpool.tile(
                            [P, QB], bf16, tag="ob", name=f"ob{rep}_{h}{M}"
                        )
                        state["osb", h, M] = osb
                        nc.scalar.activation(
                            osb[:, 0 : 2 * P], out_ps[:, 0 : 2 * P], Copy
                        )
                        nc.scalar.dma_start(
                            outT_d.ap()[h][M][:, 0 : 2 * P], osb[:, 0 : 2 * P]
                        )
                elif gp == 2 * M + 1:
                    # pair B -> slot 2M+1, live [2P, QB); host ignores [0, 2P)
                    nc.vector.tensor_copy(
                        ship[:, gp, 2 * P : 3 * P], pt[:, 2 * P : 3 * P]
                    )
                    nc.vector.tensor_add(
                        ship[:, gp, 3 * P : QB],
                        pt[:, 3 * P : QB],
                        pt[:, QB + 3 * P : 2 * QB],
                    )
                    lo_slot = 2 * M + 1 if tail_split else 2 * M
                    nc.sync.dma_start(
                        acc_d.ap()[h][M][:, lo_slot : 2 * M + 2, :],
                        ship[:, lo_slot : 2 * M + 2, :],
                    )
                else:
                    nc.vector.tensor_add(
                        ship[:, gp], pt[:, 0:QB], pt[:, QB : 2 * QB]
                    )
                    if gp == 2 * M - 1:
                        # bulk ship: all non-diag slots [0, 2M) complete
                        nc.sync.dma_start(
                            acc_d.ap()[h][M][:, 0 : 2 * M, :],
                            ship[:, 0 : 2 * M, :],
                        )

                if last:
                    if copy_split:
                        osb = state.pop(("osb", h, M))
                        nc.scalar.activation(
                            osb[:, 2 * P :], out_ps[:, 2 * P :], Copy
                        )
                        nc.scalar.dma_start(
                            outT_d.ap()[h][M][:, 2 * P :], osb[:, 2 * P :]
                        )
                    else:
                        osb = obpool.tile(
                            [P, QB], bf16, tag="ob", name=f"ob{rep}_{h}{M}"
                        )
                        nc.scalar.activation(osb[:], out_ps[:], Copy)
                        # last head's outT DMAs dispatch on the (then-idle)
                        # ACT queue so they don't head-of-line block the
                        # tail ship DMAs on SP
                        if h == HPC - 1:
                            nc.scalar.dma_start(outT_d.ap()[h][M], osb[:])
                        else:
                            nc.sync.dma_start(outT_d.ap()[h][M], osb[:])
                    del state["out_ps", h, M]
                    del state["ship", h, M]

            LOOKAHEAD = 8
            for rep in range(reps):
                state["rep"] = rep
                for i in range(min(LOOKAHEAD, len(groups))):
                    produce(i)
                for i in range(len(groups)):
                    if i + LOOKAHEAD < len(groups):
                        produce(i + LOOKAHEAD)
                    consume(i)

    nc.compile()
    return nc


def _host_bmask():
    p = np.arange(P)[:, None, None]
    t = np.arange(2)[None, :, None]
    c = np.arange(QB)[None, None, :]
    bm = np.where(c >= 128 * t + p, B_S, B_MASKED).astype(np.int16)
    return bm.reshape(P, 2 * QB)


def _pack_inputs(q, k, v):
    """Per-core input dict list (shared by kernel() and test harnesses)."""
    bm = _host_bmask()
    in_maps = []
    for c in range(NCORES):
        qT_c = np.ascontiguousarray(
            q[:, HPC * c : HPC * (c + 1), :].transpose(1, 2, 0)
        ).astype(BF16)
        kT_c = np.ascontiguousarray(k[:, c, :].T).astype(BF16)
        v_c = np.ascontiguousarray(
            v[:, c, :].reshape(NKB, P, D).transpose(1, 0, 2).reshape(P, SEQ)
        ).astype(BF16)
        in_maps.append({"qT": qT_c, "kT": kT_c, "v": v_c, "bmask": bm})
    return in_maps


def kernel(q, k, v, k_cache=None, v_cache=None, slot_mapping=None, **_):
    from concourse.bass_utils import run_bass_kernel_spmd

    if "nc" not in _COMPILED:
        _COMPILED["nc"] = _build()
    nc = _COMPILED["nc"]

    q = np.asarray(q, dtype=np.float32)
    k = np.asarray(k, dtype=np.float32)
    v = np.asarray(v, dtype=np.float32)

    in_maps = _pack_inputs(q, k, v)
    res = run_bass_kernel_spmd(nc, in_maps, list(range(NCORES)))

    out = np.empty((SEQ, NUM_HEADS, D), np.float32)
    for c in range(NCORES):
        oT = res.results[c]["outT"].astype(np.float32)   # [HPC, NQB, d, q]
        ac = res.results[c]["acc"]                        # [HPC, NQB, p, slot, q]
        for h in range(HPC):
            for M in range(NQB):
                a = ac[h, M].astype(np.float32)  # [128, NSLOT, 512]
                l = a[:, 0 : 2 * M + 1, :].sum(axis=(0, 1))  # pairs + pair A
                l[2 * P :] += a[:, 2 * M + 1, 2 * P :].sum(axis=0)  # pair B
                out[M * QB : (M + 1) * QB, HPC * c + h, :] = (oT[h, M] / l).T
    return out
